# revision 10
# baseline (speedup 1.0000x reference)
"""Multi-head attention (B=2, S=2048, D=1024, H=16, Dk=64) on 8 TRN2 cores.

Sharding: core c handles batch b=c//4 and head group g=c%4 (heads 4g..4g+3,
i.e. projection output dims 256g..256g+256). Fully independent cores, no
collectives.

Device pipeline per core (all matmul inputs bf16, fp32 PSUM accumulation):
  - K/Q projections into transposed layout  QT/KT [256 dims, 2048 seq]
    (lhsT = W^T chunk, rhs = x^T chunk; two seq-chunks per weight load to
    amortize LDWEIGHTS; per-partition bias added on DVE during the
    PSUM->SBUF copy; Wq pre-scaled by 1/8 = 1/sqrt(Dk) on host).
  - V projection into natural layout VH [seq, dims] with per-head 65 cols
    (col 64 is an all-ones output dim giving the softmax denominator);
    bias + the ones column added via a broadcast tile in the DVE copy.
  - Scores computed transposed: S^T[kv,q] = KT-block (stationary) @ QT-chunk,
    a head pair sharing one [128,1024] PSUM tile. Causal: upper blocks
    skipped; diagonal blocks N-trimmed in the matmul, masked with a
    [128,128] tril-window add on DVE, exp AP trimmed to match. No
    max-subtraction (|scores| <= ~4). E in bf16.
  - PV: O'^T[65, q] += VH'-block (stationary) @ E-block, accumulated over
    kv blocks in PSUM, N-trimmed on diagonal blocks. Row 64 = sum(E).
  - O'^T copied to SBUF (DVE) and DMA'd out (GpSimd queues); final division
    + head interleave on host.
"""

import numpy as np
import ml_dtypes

B, S, D, H, DK = 2, 2048, 1024, 16, 64
N_CORES = 8
HPC = 4          # heads per core
GD = HPC * DK    # group dims = 256
W65 = HPC * 65   # V-projection output cols (64 data + 1 ones per head)
QC = 512         # q-chunk (also seq projection chunk)
N_QC = S // QC   # 4
N_KB = S // 128  # 16
NKC = D // 128   # 8 contraction chunks
bf16 = ml_dtypes.bfloat16

_cache: dict = {}


def _build(mode: str):
    """mode: 'causal' (diag-window masks, upper blocks skipped),
    'none' (no masking, all blocks), 'general' (per-block masks from DRAM)."""
    import concourse.bass as bass
    import concourse.mybir as mybir
    from concourse import bacc
    from concourse.tile import TileContext

    fp32 = mybir.dt.float32
    bf = mybir.dt.bfloat16
    AF = mybir.ActivationFunctionType

    nc = bacc.Bacc("TRN2", target_bir_lowering=False, debug=False,
                   num_devices=N_CORES)

    # host-prepacked inputs (see kernel() below)
    xq = nc.dram_tensor("xq", [NKC, 128, S], bf, kind="ExternalInput")
    xk = nc.dram_tensor("xk", [NKC, 128, S], bf, kind="ExternalInput")
    xv = nc.dram_tensor("xv", [NKC, 128, S], bf, kind="ExternalInput")
    wq = nc.dram_tensor("wq", [128, NKC * GD], bf, kind="ExternalInput")
    wk = nc.dram_tensor("wk", [128, NKC * GD], bf, kind="ExternalInput")
    wv = nc.dram_tensor("wv", [128, NKC * W65], bf, kind="ExternalInput")
    vb = nc.dram_tensor("vb", [128, W65], bf, kind="ExternalInput")
    bqk = nc.dram_tensor("bqk", [128, 4], fp32, kind="ExternalInput")
    if mode == "causal":
        cmw = nc.dram_tensor("cmw", [128, 128], bf, kind="ExternalInput")
    elif mode == "general":
        amaskT = nc.dram_tensor("amaskT", [S, S], bf, kind="ExternalInput")
    out = nc.dram_tensor("out", [HPC, 65, S], fp32, kind="ExternalOutput")

    HS = S // 2  # DMA half

    with TileContext(nc) as tc:
        with (
            tc.tile_pool(name="res", bufs=1) as res,
            tc.tile_pool(name="mload", bufs=4) as mload,
            tc.tile_pool(name="eload", bufs=6) as eload,
            tc.tile_pool(name="oout", bufs=6) as oout,
            tc.tile_pool(name="pproj", bufs=2, space="PSUM") as pproj,
            tc.tile_pool(name="pscore", bufs=2, space="PSUM") as pscore,
            tc.tile_pool(name="ppv", bufs=2, space="PSUM") as ppv,
        ):
            # ---- resident loads: K-path first so PE unblocks fastest ----
            xk_s = res.tile([128, NKC * S], bf, tag="xk")
            xq_s = res.tile([128, NKC * S], bf, tag="xq")
            xv_s = res.tile([128, NKC * S], bf, tag="xv")
            wq_s = res.tile([128, NKC * GD], bf, tag="wq")
            wk_s = res.tile([128, NKC * GD], bf, tag="wk")
            wv_s = res.tile([128, NKC * W65], bf, tag="wv")
            vb_s = res.tile([128, W65], bf, tag="vb")
            bqk_s = res.tile([128, 4], fp32, tag="bqk")

            def xhalf(dst, src, kc, h):
                nc.sync.dma_start(
                    dst[:, kc * S + h * HS: kc * S + (h + 1) * HS],
                    src[kc, :, h * HS:(h + 1) * HS])

            nc.sync.dma_start(wk_s[:], wk[:, :])
            nc.sync.dma_start(bqk_s[:], bqk[:, :])
            for kc in range(NKC):
                xhalf(xk_s, xk, kc, 0)
            nc.sync.dma_start(wq_s[:], wq[:, :])
            for kc in range(NKC):
                xhalf(xq_s, xq, kc, 0)
            nc.sync.dma_start(wv_s[:], wv[:, :])
            nc.gpsimd.dma_start(vb_s[:], vb[:, :])
            for kc in range(NKC):
                xhalf(xv_s, xv, kc, 0)
            for kc in range(NKC):
                xhalf(xk_s, xk, kc, 1)
            for kc in range(NKC):
                xhalf(xq_s, xq, kc, 1)
            for kc in range(NKC):
                xhalf(xv_s, xv, kc, 1)
            if mode == "causal":
                cmw_s = res.tile([128, 128], bf, tag="cmw")
                nc.gpsimd.dma_start(cmw_s[:], cmw[:, :])

            # resident projected activations
            qt_s = [res.tile([128, S], bf, tag=f"qt{m}", name=f"qt{m}")
                    for m in range(2)]
            kt_s = [res.tile([128, S], bf, tag=f"kt{m}", name=f"kt{m}")
                    for m in range(2)]
            vh_s = res.tile([128, N_KB * W65], bf, tag="vh")

            def attention(sc, filler=None, fill_rate=0):
                cs = slice(sc * QC, (sc + 1) * QC)
                n_kb = 4 * sc + 4 if mode == "causal" else N_KB

                def fill():
                    if filler is None:
                        return
                    for _ in range(fill_rate):
                        try:
                            next(filler)
                        except StopIteration:
                            return

                for p in range(2):
                    pv = [ppv.tile([65, QC], fp32, tag="pv", name=f"pv{b2}")
                          for b2 in range(2)]
                    for kb in range(n_kb):
                        j = kb - 4 * sc if mode == "causal" else -1
                        t = 128 * j if j > 0 else 0  # trimmed leading cols
                        st = pscore.tile([128, 2 * QC], fp32, tag="s")
                        for b2 in range(2):
                            nc.tensor.matmul(
                                st[:, b2 * QC + t:(b2 + 1) * QC],
                                kt_s[p][b2 * 64:(b2 + 1) * 64,
                                        kb * 128:(kb + 1) * 128],
                                qt_s[p][b2 * 64:(b2 + 1) * 64,
                                        sc * QC + t:(sc + 1) * QC],
                                start=True, stop=True)
                        et = eload.tile([128, 2 * QC], bf, tag="e")
                        if j >= 0:
                            for b2 in range(2):
                                nc.vector.tensor_add(
                                    st[:, b2 * QC + t: b2 * QC + t + 128],
                                    st[:, b2 * QC + t: b2 * QC + t + 128],
                                    cmw_s[:])
                        if t > 0:
                            st3 = st[:].rearrange("p (h n) -> p h n", h=2)
                            et3 = et[:].rearrange("p (h n) -> p h n", h=2)
                            nc.scalar.activation(et3[:, :, t:], st3[:, :, t:],
                                                 AF.Exp)
                        else:
                            if mode == "general":
                                mt = mload.tile([128, QC], bf, tag="mt")
                                nc.sync.dma_start(
                                    mt[:], amaskT[kb * 128:(kb + 1) * 128, cs])
                                for b2 in range(2):
                                    nc.vector.tensor_add(
                                        st[:, b2 * QC:(b2 + 1) * QC],
                                        st[:, b2 * QC:(b2 + 1) * QC], mt[:])
                            nc.scalar.activation(et[:], st[:], AF.Exp)
                        for b2 in range(2):
                            h = 2 * p + b2
                            nc.tensor.matmul(
                                pv[b2][:, t:],
                                vh_s[:, kb * W65 + h * 65:
                                        kb * W65 + h * 65 + 65],
                                et[:, b2 * QC + t:(b2 + 1) * QC],
                                start=(kb == 0), stop=(kb == n_kb - 1))
                        fill()
                    for b2 in range(2):
                        h = 2 * p + b2
                        ot = oout.tile([65, QC], fp32, tag="o")
                        nc.vector.tensor_copy(ot[:], pv[b2][:])
                        nc.gpsimd.dma_start(out[h, :, cs], ot[:])

            def proj_pair(scp):
                """Generator: K/Q/V projections for chunks 2scp, 2scp+1,
                yielding between accumulation steps for interleaving."""
                sc0, sc1 = 2 * scp, 2 * scp + 1
                for w_s, x_s, dst, bcol in ((wk_s, xk_s, kt_s, 2),
                                            (wq_s, xq_s, qt_s, 0)):
                    for m in range(2):
                        psA = pproj.tile([128, QC], fp32, tag="proj",
                                         name="psA")
                        psB = pproj.tile([128, QC], fp32, tag="proj",
                                         name="psB")
                        for kc in range(NKC):
                            wsl = w_s[:, kc * GD + m * 128:
                                      kc * GD + (m + 1) * 128]
                            nc.tensor.matmul(
                                psA[:], wsl,
                                x_s[:, kc * S + sc0 * QC:
                                       kc * S + (sc0 + 1) * QC],
                                start=(kc == 0), stop=(kc == NKC - 1))
                            nc.tensor.matmul(
                                psB[:], wsl,
                                x_s[:, kc * S + sc1 * QC:
                                       kc * S + (sc1 + 1) * QC],
                                start=(kc == 0), stop=(kc == NKC - 1))
                            yield
                        bias = bqk_s[:, bcol + m:bcol + m + 1]
                        nc.vector.tensor_scalar_add(
                            dst[m][:, sc0 * QC:(sc0 + 1) * QC], psA[:], bias)
                        nc.vector.tensor_scalar_add(
                            dst[m][:, sc1 * QC:(sc1 + 1) * QC], psB[:], bias)
                        yield
                for sb in range(sc0 * 4, (sc1 + 1) * 4):
                    so = sb * 128
                    ps = pproj.tile([128, W65], fp32, tag="proj")
                    for kc in range(NKC):
                        nc.tensor.matmul(
                            ps[:],
                            xv_s[:, kc * S + so: kc * S + so + 128],
                            wv_s[:, kc * W65:(kc + 1) * W65],
                            start=(kc == 0), stop=(kc == NKC - 1))
                        if kc % 4 == 3:
                            yield
                    nc.vector.tensor_add(vh_s[:, sb * W65:(sb + 1) * W65],
                                         ps[:], vb_s[:])

            # chunks 0/1 projected eagerly; chunks 2/3 interleaved into the
            # attention of chunks 0/1 to fill PE stalls caused by exp latency
            for _ in proj_pair(0):
                pass
            g1 = proj_pair(1)
            attention(0, filler=g1, fill_rate=4)
            attention(1, filler=g1, fill_rate=2)
            for _ in g1:
                pass
            attention(2)
            attention(3)

    nc.compile()
    return nc


def _get_nc(mode: str):
    if mode not in _cache:
        _cache[mode] = _build(mode)
    return _cache[mode]


def kernel(q, k, v, mask, Wq, bq, Wk, bk, Wv, bv):
    q = np.asarray(q, np.float32)
    k = np.asarray(k, np.float32)
    v = np.asarray(v, np.float32)
    Wq = np.asarray(Wq, np.float32)
    Wk = np.asarray(Wk, np.float32)
    Wv = np.asarray(Wv, np.float32)
    bq = np.asarray(bq, np.float32)
    bk = np.asarray(bk, np.float32)
    bv = np.asarray(bv, np.float32)
    m2 = np.asarray(mask)[0, 0]

    causal = bool(np.array_equal(m2 != 0, np.tril(np.ones((S, S), bool))))
    if causal:
        mode = "causal"
    elif np.all(m2 != 0):
        mode = "none"
    else:
        mode = "general"

    from concourse.bass_utils import run_bass_kernel_spmd

    nc = _get_nc(mode)

    in_maps = []
    for c in range(N_CORES):
        b, g = divmod(c, HPC)
        gsl = slice(g * GD, (g + 1) * GD)
        # V weights: per head 65 cols (64 data + zero col for the ones dim);
        # the ones + bias come from the broadcast add tile vb.
        wv65 = np.zeros((D, W65), np.float32)
        vbrow = np.zeros((1, W65), np.float32)
        for h in range(HPC):
            wv65[:, h * 65:h * 65 + 64] = Wv[g * GD + h * DK:
                                             g * GD + h * DK + DK, :].T
            vbrow[0, h * 65:h * 65 + 64] = bv[g * GD + h * DK:
                                              g * GD + h * DK + DK]
            vbrow[0, h * 65 + 64] = 1.0

        def packw(wt):
            n = wt.shape[1]
            return np.ascontiguousarray(
                wt.reshape(NKC, 128, n).transpose(1, 0, 2).reshape(128, NKC * n)
            ).astype(bf16)

        im = {
            "xq": np.ascontiguousarray(q[b].T.reshape(NKC, 128, S)).astype(bf16),
            "xk": np.ascontiguousarray(k[b].T.reshape(NKC, 128, S)).astype(bf16),
            "xv": np.ascontiguousarray(v[b].T.reshape(NKC, 128, S)).astype(bf16),
            "wq": packw(Wq[gsl, :].T / 8.0),
            "wk": packw(Wk[gsl, :].T),
            "wv": packw(wv65),
            "vb": np.broadcast_to(vbrow, (128, W65)).astype(bf16).copy(),
            "bqk": np.stack([bq[gsl][:128] / 8.0, bq[gsl][128:] / 8.0,
                             bk[gsl][:128], bk[gsl][128:]], 1)
                     .astype(np.float32).copy(),
        }
        if mode == "causal":
            r = np.arange(128)[:, None]
            cc = np.arange(128)[None, :]
            im["cmw"] = np.where(r <= cc, 0.0, -1e9).astype(bf16)
        elif mode == "general":
            add = np.where(m2 == 0, -1e9, 0.0).astype(np.float32)
            im["amaskT"] = add.T.astype(bf16).copy()
        in_maps.append(im)

    global _last_in_maps
    _last_in_maps = in_maps
    res = run_bass_kernel_spmd(nc, in_maps, core_ids=list(range(N_CORES)))

    outf = np.empty((B, S, D), np.float32)
    for c in range(N_CORES):
        b, g = divmod(c, HPC)
        o = res.results[c]["out"]  # [HPC, 65, S]
        num = o[:, :64, :]         # [HPC, 64, S]
        den = o[:, 64:65, :]       # [HPC, 1, S]
        oh = num / den             # [HPC, 64, S]
        outf[b, :, g * GD:(g + 1) * GD] = (
            oh.transpose(2, 0, 1).reshape(S, GD))
    return outf


# revision 11
# speedup vs baseline: 1.2164x; 1.2164x over previous
"""Multi-head attention (B=2, S=2048, D=1024, H=16, Dk=64) on 8 TRN2 cores.

Sharding: core c handles batch b=c//4 and head group g=c%4 (heads 4g..4g+3,
i.e. projection output dims 256g..256g+256). Fully independent cores, no
collectives.

Device pipeline per core (all matmul inputs bf16, fp32 PSUM accumulation):
  - K/Q projections into transposed layout  QT/KT [256 dims, 2048 seq]
    (lhsT = W^T chunk, rhs = x^T chunk; two seq-chunks per weight load to
    amortize LDWEIGHTS; per-partition bias added on DVE during the
    PSUM->SBUF copy; Wq pre-scaled by 1/8 = 1/sqrt(Dk) on host).
  - V projection into natural layout VH [seq, dims] with per-head 65 cols
    (col 64 is an all-ones output dim giving the softmax denominator);
    bias + the ones column added via a broadcast tile in the DVE copy.
  - Scores computed transposed: S^T[kv,q] = KT-block (stationary) @ QT-chunk,
    a head pair sharing one [128,1024] PSUM tile. Causal: upper blocks
    skipped; diagonal blocks N-trimmed in the matmul, masked with a
    [128,128] tril-window add on DVE, exp AP trimmed to match. No
    max-subtraction (|scores| <= ~4). E in bf16.
  - PV: O'^T[65, q] += VH'-block (stationary) @ E-block, accumulated over
    kv blocks in PSUM, N-trimmed on diagonal blocks. Row 64 = sum(E).
  - O'^T copied to SBUF (DVE) and DMA'd out (GpSimd queues); final division
    + head interleave on host.
"""

import numpy as np
import ml_dtypes

B, S, D, H, DK = 2, 2048, 1024, 16, 64
N_CORES = 8
HPC = 4          # heads per core
GD = HPC * DK    # group dims = 256
W65 = HPC * 65   # V-projection output cols (64 data + 1 ones per head)
QC = 512         # q-chunk (also seq projection chunk)
N_QC = S // QC   # 4
N_KB = S // 128  # 16
NKC = D // 128   # 8 contraction chunks
bf16 = ml_dtypes.bfloat16

_cache: dict = {}


def _build(mode: str):
    """mode: 'causal' (diag-window masks, upper blocks skipped),
    'none' (no masking, all blocks), 'general' (per-block masks from DRAM)."""
    import concourse.bass as bass
    import concourse.mybir as mybir
    from concourse import bacc
    from concourse.tile import TileContext

    fp32 = mybir.dt.float32
    bf = mybir.dt.bfloat16
    AF = mybir.ActivationFunctionType

    nc = bacc.Bacc("TRN2", target_bir_lowering=False, debug=False,
                   num_devices=N_CORES)

    # host-prepacked inputs (see kernel() below)
    xq = nc.dram_tensor("xq", [NKC, 128, S], bf, kind="ExternalInput")
    xk = nc.dram_tensor("xk", [NKC, 128, S], bf, kind="ExternalInput")
    xv = nc.dram_tensor("xv", [NKC, 128, S], bf, kind="ExternalInput")
    wq = nc.dram_tensor("wq", [128, NKC * GD], bf, kind="ExternalInput")
    wk = nc.dram_tensor("wk", [128, NKC * GD], bf, kind="ExternalInput")
    wv = nc.dram_tensor("wv", [128, NKC * W65], bf, kind="ExternalInput")
    vb = nc.dram_tensor("vb", [128, W65], bf, kind="ExternalInput")
    bqk = nc.dram_tensor("bqk", [128, 4], fp32, kind="ExternalInput")
    if mode == "causal":
        cmw = nc.dram_tensor("cmw", [128, 128], bf, kind="ExternalInput")
    elif mode == "general":
        amaskT = nc.dram_tensor("amaskT", [S, S], bf, kind="ExternalInput")
    out = nc.dram_tensor("out", [HPC, 65, S], fp32, kind="ExternalOutput")

    HS = S // 2  # DMA half

    with TileContext(nc) as tc:
        with (
            tc.tile_pool(name="res", bufs=1) as res,
            tc.tile_pool(name="mload", bufs=4) as mload,
            tc.tile_pool(name="eload", bufs=6) as eload,
            tc.tile_pool(name="oout", bufs=6) as oout,
            tc.tile_pool(name="pproj", bufs=2, space="PSUM") as pproj,
            tc.tile_pool(name="pscore", bufs=2, space="PSUM") as pscore,
            tc.tile_pool(name="ppv", bufs=2, space="PSUM") as ppv,
        ):
            # ---- resident loads: K-path first so PE unblocks fastest ----
            xk_s = res.tile([128, NKC * S], bf, tag="xk")
            xq_s = res.tile([128, NKC * S], bf, tag="xq")
            xv_s = res.tile([128, NKC * S], bf, tag="xv")
            wq_s = res.tile([128, NKC * GD], bf, tag="wq")
            wk_s = res.tile([128, NKC * GD], bf, tag="wk")
            wv_s = res.tile([128, NKC * W65], bf, tag="wv")
            vb_s = res.tile([128, W65], bf, tag="vb")
            bqk_s = res.tile([128, 4], fp32, tag="bqk")

            def xhalf(dst, src, kc, h):
                nc.sync.dma_start(
                    dst[:, kc * S + h * HS: kc * S + (h + 1) * HS],
                    src[kc, :, h * HS:(h + 1) * HS])

            nc.sync.dma_start(wk_s[:], wk[:, :])
            nc.sync.dma_start(bqk_s[:], bqk[:, :])
            for kc in range(NKC):
                xhalf(xk_s, xk, kc, 0)
            nc.sync.dma_start(wq_s[:], wq[:, :])
            for kc in range(NKC):
                xhalf(xq_s, xq, kc, 0)
            nc.sync.dma_start(wv_s[:], wv[:, :])
            nc.gpsimd.dma_start(vb_s[:], vb[:, :])
            for kc in range(NKC):
                xhalf(xv_s, xv, kc, 0)
            for kc in range(NKC):
                xhalf(xk_s, xk, kc, 1)
            for kc in range(NKC):
                xhalf(xq_s, xq, kc, 1)
            for kc in range(NKC):
                xhalf(xv_s, xv, kc, 1)
            if mode == "causal":
                cmw_s = res.tile([128, 128], bf, tag="cmw")
                nc.gpsimd.dma_start(cmw_s[:], cmw[:, :])

            # resident projected activations
            qt_s = [res.tile([128, S], bf, tag=f"qt{m}", name=f"qt{m}")
                    for m in range(2)]
            kt_s = [res.tile([128, S], bf, tag=f"kt{m}", name=f"kt{m}")
                    for m in range(2)]
            vh_s = res.tile([128, N_KB * W65], bf, tag="vh")

            def attention(sc, filler=None, fill_rate=0):
                cs = slice(sc * QC, (sc + 1) * QC)
                n_kb = 4 * sc + 4 if mode == "causal" else N_KB

                def fill():
                    if filler is None:
                        return
                    for _ in range(fill_rate):
                        try:
                            next(filler)
                        except StopIteration:
                            return

                for p in range(2):
                    pv = [ppv.tile([65, QC], fp32, tag="pv", name=f"pv{b2}")
                          for b2 in range(2)]
                    for kb in range(n_kb):
                        j = kb - 4 * sc if mode == "causal" else -1
                        t = 128 * j if j > 0 else 0  # trimmed leading cols
                        st = pscore.tile([128, 2 * QC], fp32, tag="s")
                        for b2 in range(2):
                            nc.tensor.matmul(
                                st[:, b2 * QC + t:(b2 + 1) * QC],
                                kt_s[p][b2 * 64:(b2 + 1) * 64,
                                        kb * 128:(kb + 1) * 128],
                                qt_s[p][b2 * 64:(b2 + 1) * 64,
                                        sc * QC + t:(sc + 1) * QC],
                                start=True, stop=True)
                        et = eload.tile([128, 2 * QC], bf, tag="e")
                        if j >= 0:
                            for b2 in range(2):
                                nc.vector.tensor_add(
                                    st[:, b2 * QC + t: b2 * QC + t + 128],
                                    st[:, b2 * QC + t: b2 * QC + t + 128],
                                    cmw_s[:])
                        if t > 0:
                            st3 = st[:].rearrange("p (h n) -> p h n", h=2)
                            et3 = et[:].rearrange("p (h n) -> p h n", h=2)
                            nc.scalar.activation(et3[:, :, t:], st3[:, :, t:],
                                                 AF.Exp)
                        else:
                            if mode == "general":
                                mt = mload.tile([128, QC], bf, tag="mt")
                                nc.sync.dma_start(
                                    mt[:], amaskT[kb * 128:(kb + 1) * 128, cs])
                                for b2 in range(2):
                                    nc.vector.tensor_add(
                                        st[:, b2 * QC:(b2 + 1) * QC],
                                        st[:, b2 * QC:(b2 + 1) * QC], mt[:])
                            nc.scalar.activation(et[:], st[:], AF.Exp)
                        for b2 in range(2):
                            h = 2 * p + b2
                            nc.tensor.matmul(
                                pv[b2][:, t:],
                                vh_s[:, kb * W65 + h * 65:
                                        kb * W65 + h * 65 + 65],
                                et[:, b2 * QC + t:(b2 + 1) * QC],
                                start=(kb == 0), stop=(kb == n_kb - 1))
                        fill()
                    for b2 in range(2):
                        h = 2 * p + b2
                        ot = oout.tile([65, QC], fp32, tag="o")
                        nc.vector.tensor_copy(ot[:], pv[b2][:])
                        nc.gpsimd.dma_start(out[h, :, cs], ot[:])

            def proj_pair(scp):
                """Generator: K/Q/V projections for chunks 2scp, 2scp+1,
                yielding between accumulation steps for interleaving."""
                sc0, sc1 = 2 * scp, 2 * scp + 1
                for w_s, x_s, dst, bcol in ((wk_s, xk_s, kt_s, 2),
                                            (wq_s, xq_s, qt_s, 0)):
                    for m in range(2):
                        psA = pproj.tile([128, QC], fp32, tag="proj",
                                         name="psA")
                        psB = pproj.tile([128, QC], fp32, tag="proj",
                                         name="psB")
                        for kc in range(NKC):
                            wsl = w_s[:, kc * GD + m * 128:
                                      kc * GD + (m + 1) * 128]
                            nc.tensor.matmul(
                                psA[:], wsl,
                                x_s[:, kc * S + sc0 * QC:
                                       kc * S + (sc0 + 1) * QC],
                                start=(kc == 0), stop=(kc == NKC - 1))
                            nc.tensor.matmul(
                                psB[:], wsl,
                                x_s[:, kc * S + sc1 * QC:
                                       kc * S + (sc1 + 1) * QC],
                                start=(kc == 0), stop=(kc == NKC - 1))
                            yield
                        bias = bqk_s[:, bcol + m:bcol + m + 1]
                        nc.vector.tensor_scalar_add(
                            dst[m][:, sc0 * QC:(sc0 + 1) * QC], psA[:], bias)
                        nc.vector.tensor_scalar_add(
                            dst[m][:, sc1 * QC:(sc1 + 1) * QC], psB[:], bias)
                        yield
                for sb in range(sc0 * 4, (sc1 + 1) * 4):
                    so = sb * 128
                    ps = pproj.tile([128, W65], fp32, tag="proj")
                    for kc in range(NKC):
                        nc.tensor.matmul(
                            ps[:],
                            xv_s[:, kc * S + so: kc * S + so + 128],
                            wv_s[:, kc * W65:(kc + 1) * W65],
                            start=(kc == 0), stop=(kc == NKC - 1))
                        if kc % 4 == 3:
                            yield
                    nc.vector.tensor_add(vh_s[:, sb * W65:(sb + 1) * W65],
                                         ps[:], vb_s[:])

            # chunks 0/1 projected eagerly; chunks 2/3 interleaved into the
            # attention of chunks 0/1 to fill PE stalls caused by exp latency
            for _ in proj_pair(0):
                pass
            attention(0)
            attention(1)
            for _ in proj_pair(1):
                pass
            attention(2)
            attention(3)

    nc.compile()
    return nc


def _get_nc(mode: str):
    if mode not in _cache:
        _cache[mode] = _build(mode)
    return _cache[mode]


def kernel(q, k, v, mask, Wq, bq, Wk, bk, Wv, bv):
    q = np.asarray(q, np.float32)
    k = np.asarray(k, np.float32)
    v = np.asarray(v, np.float32)
    Wq = np.asarray(Wq, np.float32)
    Wk = np.asarray(Wk, np.float32)
    Wv = np.asarray(Wv, np.float32)
    bq = np.asarray(bq, np.float32)
    bk = np.asarray(bk, np.float32)
    bv = np.asarray(bv, np.float32)
    m2 = np.asarray(mask)[0, 0]

    causal = bool(np.array_equal(m2 != 0, np.tril(np.ones((S, S), bool))))
    if causal:
        mode = "causal"
    elif np.all(m2 != 0):
        mode = "none"
    else:
        mode = "general"

    from concourse.bass_utils import run_bass_kernel_spmd

    nc = _get_nc(mode)

    in_maps = []
    for c in range(N_CORES):
        b, g = divmod(c, HPC)
        gsl = slice(g * GD, (g + 1) * GD)
        # V weights: per head 65 cols (64 data + zero col for the ones dim);
        # the ones + bias come from the broadcast add tile vb.
        wv65 = np.zeros((D, W65), np.float32)
        vbrow = np.zeros((1, W65), np.float32)
        for h in range(HPC):
            wv65[:, h * 65:h * 65 + 64] = Wv[g * GD + h * DK:
                                             g * GD + h * DK + DK, :].T
            vbrow[0, h * 65:h * 65 + 64] = bv[g * GD + h * DK:
                                              g * GD + h * DK + DK]
            vbrow[0, h * 65 + 64] = 1.0

        def packw(wt):
            n = wt.shape[1]
            return np.ascontiguousarray(
                wt.reshape(NKC, 128, n).transpose(1, 0, 2).reshape(128, NKC * n)
            ).astype(bf16)

        im = {
            "xq": np.ascontiguousarray(q[b].T.reshape(NKC, 128, S)).astype(bf16),
            "xk": np.ascontiguousarray(k[b].T.reshape(NKC, 128, S)).astype(bf16),
            "xv": np.ascontiguousarray(v[b].T.reshape(NKC, 128, S)).astype(bf16),
            "wq": packw(Wq[gsl, :].T / 8.0),
            "wk": packw(Wk[gsl, :].T),
            "wv": packw(wv65),
            "vb": np.broadcast_to(vbrow, (128, W65)).astype(bf16).copy(),
            "bqk": np.stack([bq[gsl][:128] / 8.0, bq[gsl][128:] / 8.0,
                             bk[gsl][:128], bk[gsl][128:]], 1)
                     .astype(np.float32).copy(),
        }
        if mode == "causal":
            r = np.arange(128)[:, None]
            cc = np.arange(128)[None, :]
            im["cmw"] = np.where(r <= cc, 0.0, -1e9).astype(bf16)
        elif mode == "general":
            add = np.where(m2 == 0, -1e9, 0.0).astype(np.float32)
            im["amaskT"] = add.T.astype(bf16).copy()
        in_maps.append(im)

    global _last_in_maps
    _last_in_maps = in_maps
    res = run_bass_kernel_spmd(nc, in_maps, core_ids=list(range(N_CORES)))

    outf = np.empty((B, S, D), np.float32)
    for c in range(N_CORES):
        b, g = divmod(c, HPC)
        o = res.results[c]["out"]  # [HPC, 65, S]
        num = o[:, :64, :]         # [HPC, 64, S]
        den = o[:, 64:65, :]       # [HPC, 1, S]
        oh = num / den             # [HPC, 64, S]
        outf[b, :, g * GD:(g + 1) * GD] = (
            oh.transpose(2, 0, 1).reshape(S, GD))
    return outf


# revision 13
# speedup vs baseline: 1.2391x; 1.0186x over previous
"""Multi-head attention (B=2, S=2048, D=1024, H=16, Dk=64) on 8 TRN2 cores.

Sharding: core c handles batch b=c//4 and head group g=c%4 (heads 4g..4g+3,
i.e. projection output dims 256g..256g+256). Fully independent cores, no
collectives.

Device pipeline per core (all matmul inputs bf16, fp32 PSUM accumulation):
  - K/Q projections into transposed layout  QT/KT [256 dims, 2048 seq]
    (lhsT = W^T chunk, rhs = x^T chunk; two seq-chunks per weight load to
    amortize LDWEIGHTS; per-partition bias added on DVE during the
    PSUM->SBUF copy; Wq pre-scaled by 1/8 = 1/sqrt(Dk) on host).
  - V projection into natural layout VH [seq, dims] with per-head 65 cols
    (col 64 is an all-ones output dim giving the softmax denominator);
    bias + the ones column added via a broadcast tile in the DVE copy.
  - Scores computed transposed: S^T[kv,q] = KT-block (stationary) @ QT-chunk,
    a head pair sharing one [128,1024] PSUM tile. Causal: upper blocks
    skipped; diagonal blocks N-trimmed in the matmul, masked with a
    [128,128] tril-window add on DVE, exp AP trimmed to match. No
    max-subtraction (|scores| <= ~4). E in bf16.
  - PV: O'^T[65, q] += VH'-block (stationary) @ E-block, accumulated over
    kv blocks in PSUM, N-trimmed on diagonal blocks. Row 64 = sum(E).
  - O'^T copied to SBUF (DVE) and DMA'd out (GpSimd queues); final division
    + head interleave on host.
"""

import numpy as np
import ml_dtypes

B, S, D, H, DK = 2, 2048, 1024, 16, 64
N_CORES = 8
HPC = 4          # heads per core
GD = HPC * DK    # group dims = 256
W65 = HPC * 65   # V-projection output cols (64 data + 1 ones per head)
QC = 512         # q-chunk (also seq projection chunk)
N_QC = S // QC   # 4
N_KB = S // 128  # 16
NKC = D // 128   # 8 contraction chunks
bf16 = ml_dtypes.bfloat16

_cache: dict = {}


def _build(mode: str):
    """mode: 'causal' (diag-window masks, upper blocks skipped),
    'none' (no masking, all blocks), 'general' (per-block masks from DRAM)."""
    import concourse.bass as bass
    import concourse.mybir as mybir
    from concourse import bacc
    from concourse.tile import TileContext

    fp32 = mybir.dt.float32
    bf = mybir.dt.bfloat16
    AF = mybir.ActivationFunctionType

    nc = bacc.Bacc("TRN2", target_bir_lowering=False, debug=False,
                   num_devices=N_CORES)

    # host-prepacked inputs (see kernel() below)
    xq = nc.dram_tensor("xq", [NKC, 128, S], bf, kind="ExternalInput")
    xk = nc.dram_tensor("xk", [NKC, 128, S], bf, kind="ExternalInput")
    xv = nc.dram_tensor("xv", [NKC, 128, S], bf, kind="ExternalInput")
    wq = nc.dram_tensor("wq", [128, NKC * GD], bf, kind="ExternalInput")
    wk = nc.dram_tensor("wk", [128, NKC * GD], bf, kind="ExternalInput")
    wv = nc.dram_tensor("wv", [128, NKC * W65], bf, kind="ExternalInput")
    vb = nc.dram_tensor("vb", [128, W65], bf, kind="ExternalInput")
    bqk = nc.dram_tensor("bqk", [128, 4], fp32, kind="ExternalInput")
    if mode == "causal":
        cmw = nc.dram_tensor("cmw", [128, 128], bf, kind="ExternalInput")
    elif mode == "general":
        amaskT = nc.dram_tensor("amaskT", [S, S], bf, kind="ExternalInput")
    out = nc.dram_tensor("out", [HPC, 65, S], fp32, kind="ExternalOutput")

    HS = S // 2  # DMA half

    with TileContext(nc) as tc:
        with (
            tc.tile_pool(name="res", bufs=1) as res,
            tc.tile_pool(name="mload", bufs=4) as mload,
            tc.tile_pool(name="eload", bufs=6) as eload,
            tc.tile_pool(name="oout", bufs=6) as oout,
            tc.tile_pool(name="pproj", bufs=2, space="PSUM") as pproj,
            tc.tile_pool(name="pscore", bufs=2, space="PSUM") as pscore,
            tc.tile_pool(name="ppv", bufs=2, space="PSUM") as ppv,
        ):
            # ---- resident loads: K-path first so PE unblocks fastest ----
            xk_s = res.tile([128, NKC * S], bf, tag="xk")
            xq_s = res.tile([128, NKC * S], bf, tag="xq")
            xv_s = res.tile([128, NKC * S], bf, tag="xv")
            wq_s = res.tile([128, NKC * GD], bf, tag="wq")
            wk_s = res.tile([128, NKC * GD], bf, tag="wk")
            wv_s = res.tile([128, NKC * W65], bf, tag="wv")
            vb_s = res.tile([128, W65], bf, tag="vb")
            bqk_s = res.tile([128, 4], fp32, tag="bqk")

            def xhalf(dst, src, kc, h):
                nc.sync.dma_start(
                    dst[:, kc * S + h * HS: kc * S + (h + 1) * HS],
                    src[kc, :, h * HS:(h + 1) * HS])

            nc.sync.dma_start(wk_s[:], wk[:, :])
            nc.sync.dma_start(bqk_s[:], bqk[:, :])
            for kc in range(NKC):
                xhalf(xk_s, xk, kc, 0)
            nc.sync.dma_start(wq_s[:], wq[:, :])
            for kc in range(NKC):
                xhalf(xq_s, xq, kc, 0)
            nc.sync.dma_start(wv_s[:], wv[:, :])
            nc.gpsimd.dma_start(vb_s[:], vb[:, :])
            for kc in range(NKC):
                xhalf(xv_s, xv, kc, 0)
            for kc in range(NKC):
                xhalf(xk_s, xk, kc, 1)
            for kc in range(NKC):
                xhalf(xq_s, xq, kc, 1)
            for kc in range(NKC):
                xhalf(xv_s, xv, kc, 1)
            if mode == "causal":
                cmw_s = res.tile([128, 128], bf, tag="cmw")
                nc.gpsimd.dma_start(cmw_s[:], cmw[:, :])

            # resident projected activations
            qt_s = [res.tile([128, S], bf, tag=f"qt{m}", name=f"qt{m}")
                    for m in range(2)]
            kt_s = [res.tile([128, S], bf, tag=f"kt{m}", name=f"kt{m}")
                    for m in range(2)]
            vh_s = res.tile([128, N_KB * W65], bf, tag="vh")

            def attention(sc, filler=None, fill_rate=1):
                cs = slice(sc * QC, (sc + 1) * QC)
                n_kb = 4 * sc + 4 if mode == "causal" else N_KB

                def fill():
                    if filler is None:
                        return
                    for _ in range(fill_rate):
                        try:
                            next(filler)
                        except StopIteration:
                            return

                def score_exp(p, kb):
                    j = kb - 4 * sc if mode == "causal" else -1
                    t = 128 * j if j > 0 else 0  # trimmed leading cols
                    st = pscore.tile([128, 2 * QC], fp32, tag="s")
                    for b2 in range(2):
                        nc.tensor.matmul(
                            st[:, b2 * QC + t:(b2 + 1) * QC],
                            kt_s[p][b2 * 64:(b2 + 1) * 64,
                                    kb * 128:(kb + 1) * 128],
                            qt_s[p][b2 * 64:(b2 + 1) * 64,
                                    sc * QC + t:(sc + 1) * QC],
                            start=True, stop=True)
                    et = eload.tile([128, 2 * QC], bf, tag="e")
                    if j >= 0:
                        for b2 in range(2):
                            nc.vector.tensor_add(
                                st[:, b2 * QC + t: b2 * QC + t + 128],
                                st[:, b2 * QC + t: b2 * QC + t + 128],
                                cmw_s[:])
                    if t > 0:
                        st3 = st[:].rearrange("p (h n) -> p h n", h=2)
                        et3 = et[:].rearrange("p (h n) -> p h n", h=2)
                        nc.scalar.activation(et3[:, :, t:], st3[:, :, t:],
                                             AF.Exp)
                    else:
                        if mode == "general":
                            mt = mload.tile([128, QC], bf, tag="mt")
                            nc.sync.dma_start(
                                mt[:], amaskT[kb * 128:(kb + 1) * 128, cs])
                            for b2 in range(2):
                                nc.vector.tensor_add(
                                    st[:, b2 * QC:(b2 + 1) * QC],
                                    st[:, b2 * QC:(b2 + 1) * QC], mt[:])
                        nc.scalar.activation(et[:], st[:], AF.Exp)
                    return et, t

                def pv_step(p, pv, kb, et, t):
                    for b2 in range(2):
                        h = 2 * p + b2
                        nc.tensor.matmul(
                            pv[b2][:, t:],
                            vh_s[:, kb * W65 + h * 65: kb * W65 + h * 65 + 65],
                            et[:, b2 * QC + t:(b2 + 1) * QC],
                            start=(kb == 0), stop=(kb == n_kb - 1))

                for p in range(2):
                    pv = [ppv.tile([65, QC], fp32, tag="pv", name=f"pv{b2}")
                          for b2 in range(2)]
                    # kb-pair software pipeline: scores for both kbs
                    # back-to-back (hides LDWEIGHTS), then both PV steps
                    for kb2 in range(n_kb // 2):
                        e0, t0 = score_exp(p, 2 * kb2)
                        e1, t1 = score_exp(p, 2 * kb2 + 1)
                        pv_step(p, pv, 2 * kb2, e0, t0)
                        fill()
                        pv_step(p, pv, 2 * kb2 + 1, e1, t1)
                        fill()
                    for b2 in range(2):
                        h = 2 * p + b2
                        ot = oout.tile([65, QC], fp32, tag="o")
                        nc.vector.tensor_copy(ot[:], pv[b2][:])
                        nc.gpsimd.dma_start(out[h, :, cs], ot[:])

            def proj_kq(scp):
                """Generator: K/Q projections for chunks 2scp, 2scp+1 (two
                seq-chunks per weight load), yielding after every matmul."""
                sc0, sc1 = 2 * scp, 2 * scp + 1
                for w_s, x_s, dst, bcol in ((wk_s, xk_s, kt_s, 2),
                                            (wq_s, xq_s, qt_s, 0)):
                    for m in range(2):
                        psA = pproj.tile([128, QC], fp32, tag="proj",
                                         name="psA")
                        psB = pproj.tile([128, QC], fp32, tag="proj",
                                         name="psB")
                        for kc in range(NKC):
                            wsl = w_s[:, kc * GD + m * 128:
                                      kc * GD + (m + 1) * 128]
                            nc.tensor.matmul(
                                psA[:], wsl,
                                x_s[:, kc * S + sc0 * QC:
                                       kc * S + (sc0 + 1) * QC],
                                start=(kc == 0), stop=(kc == NKC - 1))
                            yield
                            nc.tensor.matmul(
                                psB[:], wsl,
                                x_s[:, kc * S + sc1 * QC:
                                       kc * S + (sc1 + 1) * QC],
                                start=(kc == 0), stop=(kc == NKC - 1))
                            yield
                        bias = bqk_s[:, bcol + m:bcol + m + 1]
                        nc.vector.tensor_scalar_add(
                            dst[m][:, sc0 * QC:(sc0 + 1) * QC], psA[:], bias)
                        nc.vector.tensor_scalar_add(
                            dst[m][:, sc1 * QC:(sc1 + 1) * QC], psB[:], bias)
                        yield

            def proj_v(sc):
                """Generator: V projection for chunk sc, yield per matmul."""
                for sb in range(sc * 4, (sc + 1) * 4):
                    so = sb * 128
                    ps = pproj.tile([128, W65], fp32, tag="proj")
                    for kc in range(NKC):
                        nc.tensor.matmul(
                            ps[:],
                            xv_s[:, kc * S + so: kc * S + so + 128],
                            wv_s[:, kc * W65:(kc + 1) * W65],
                            start=(kc == 0), stop=(kc == NKC - 1))
                        yield
                    nc.vector.tensor_add(vh_s[:, sb * W65:(sb + 1) * W65],
                                         ps[:], vb_s[:])
                    yield

            def chain(*gens):
                for g in gens:
                    yield from g

            def drain(g):
                for _ in g:
                    pass

            # chunks 0/1 projected eagerly; K/Q of chunks 2/3 and V of chunk 2
            # trickle into the attention of chunks 0/1 (one matmul per PV
            # step) to fill PE stalls caused by exp latency; V of chunk 3
            # trickles into attention 2 (its VH only needed by attention 3).
            drain(chain(proj_kq(0), proj_v(0), proj_v(1)))
            g1 = chain(proj_kq(1), proj_v(2))
            attention(0, filler=g1)
            attention(1, filler=g1)
            drain(g1)
            g2 = proj_v(3)
            attention(2, filler=g2)
            drain(g2)
            attention(3)

    nc.compile()
    return nc


def _get_nc(mode: str):
    if mode not in _cache:
        _cache[mode] = _build(mode)
    return _cache[mode]


def kernel(q, k, v, mask, Wq, bq, Wk, bk, Wv, bv):
    q = np.asarray(q, np.float32)
    k = np.asarray(k, np.float32)
    v = np.asarray(v, np.float32)
    Wq = np.asarray(Wq, np.float32)
    Wk = np.asarray(Wk, np.float32)
    Wv = np.asarray(Wv, np.float32)
    bq = np.asarray(bq, np.float32)
    bk = np.asarray(bk, np.float32)
    bv = np.asarray(bv, np.float32)
    m2 = np.asarray(mask)[0, 0]

    causal = bool(np.array_equal(m2 != 0, np.tril(np.ones((S, S), bool))))
    if causal:
        mode = "causal"
    elif np.all(m2 != 0):
        mode = "none"
    else:
        mode = "general"

    from concourse.bass_utils import run_bass_kernel_spmd

    nc = _get_nc(mode)

    in_maps = []
    for c in range(N_CORES):
        b, g = divmod(c, HPC)
        gsl = slice(g * GD, (g + 1) * GD)
        # V weights: per head 65 cols (64 data + zero col for the ones dim);
        # the ones + bias come from the broadcast add tile vb.
        wv65 = np.zeros((D, W65), np.float32)
        vbrow = np.zeros((1, W65), np.float32)
        for h in range(HPC):
            wv65[:, h * 65:h * 65 + 64] = Wv[g * GD + h * DK:
                                             g * GD + h * DK + DK, :].T
            vbrow[0, h * 65:h * 65 + 64] = bv[g * GD + h * DK:
                                              g * GD + h * DK + DK]
            vbrow[0, h * 65 + 64] = 1.0

        def packw(wt):
            n = wt.shape[1]
            return np.ascontiguousarray(
                wt.reshape(NKC, 128, n).transpose(1, 0, 2).reshape(128, NKC * n)
            ).astype(bf16)

        im = {
            "xq": np.ascontiguousarray(q[b].T.reshape(NKC, 128, S)).astype(bf16),
            "xk": np.ascontiguousarray(k[b].T.reshape(NKC, 128, S)).astype(bf16),
            "xv": np.ascontiguousarray(v[b].T.reshape(NKC, 128, S)).astype(bf16),
            "wq": packw(Wq[gsl, :].T / 8.0),
            "wk": packw(Wk[gsl, :].T),
            "wv": packw(wv65),
            "vb": np.broadcast_to(vbrow, (128, W65)).astype(bf16).copy(),
            "bqk": np.stack([bq[gsl][:128] / 8.0, bq[gsl][128:] / 8.0,
                             bk[gsl][:128], bk[gsl][128:]], 1)
                     .astype(np.float32).copy(),
        }
        if mode == "causal":
            r = np.arange(128)[:, None]
            cc = np.arange(128)[None, :]
            im["cmw"] = np.where(r <= cc, 0.0, -1e9).astype(bf16)
        elif mode == "general":
            add = np.where(m2 == 0, -1e9, 0.0).astype(np.float32)
            im["amaskT"] = add.T.astype(bf16).copy()
        in_maps.append(im)

    global _last_in_maps
    _last_in_maps = in_maps
    res = run_bass_kernel_spmd(nc, in_maps, core_ids=list(range(N_CORES)))

    outf = np.empty((B, S, D), np.float32)
    for c in range(N_CORES):
        b, g = divmod(c, HPC)
        o = res.results[c]["out"]  # [HPC, 65, S]
        num = o[:, :64, :]         # [HPC, 64, S]
        den = o[:, 64:65, :]       # [HPC, 1, S]
        oh = num / den             # [HPC, 64, S]
        outf[b, :, g * GD:(g + 1) * GD] = (
            oh.transpose(2, 0, 1).reshape(S, GD))
    return outf


# revision 17
# speedup vs baseline: 1.2494x; 1.0083x over previous
"""Multi-head attention (B=2, S=2048, D=1024, H=16, Dk=64) on 8 TRN2 cores.

Sharding: core c handles batch b=c//4 and head group g=c%4 (heads 4g..4g+3,
i.e. projection output dims 256g..256g+256). Fully independent cores, no
collectives.

Device pipeline per core (all matmul inputs bf16, fp32 PSUM accumulation):
  - K/Q projections into transposed layout  QT/KT [256 dims, 2048 seq]
    (lhsT = W^T chunk, rhs = x^T chunk; two seq-chunks per weight load to
    amortize LDWEIGHTS; per-partition bias added on DVE during the
    PSUM->SBUF copy; Wq pre-scaled by 1/8 = 1/sqrt(Dk) on host).
  - V projection into natural layout VH [seq, dims] with per-head 65 cols
    (col 64 is an all-ones output dim giving the softmax denominator);
    bias + the ones column added via a broadcast tile in the DVE copy.
  - Scores computed transposed: S^T[kv,q] = KT-block (stationary) @ QT-chunk,
    a head pair sharing one [128,1024] PSUM tile. Causal: upper blocks
    skipped; diagonal blocks N-trimmed in the matmul, masked with a
    [128,128] tril-window add on DVE, exp AP trimmed to match. No
    max-subtraction (|scores| <= ~4). E in bf16.
  - PV: O'^T[65, q] += VH'-block (stationary) @ E-block, accumulated over
    kv blocks in PSUM, N-trimmed on diagonal blocks. Row 64 = sum(E).
  - O'^T copied to SBUF (DVE) and DMA'd out (GpSimd queues); final division
    + head interleave on host.
"""

import numpy as np
import ml_dtypes

B, S, D, H, DK = 2, 2048, 1024, 16, 64
N_CORES = 8
HPC = 4          # heads per core
GD = HPC * DK    # group dims = 256
W65 = HPC * 65   # V-projection output cols (64 data + 1 ones per head)
QC = 512         # q-chunk (also seq projection chunk)
N_QC = S // QC   # 4
N_KB = S // 128  # 16
NKC = D // 128   # 8 contraction chunks
bf16 = ml_dtypes.bfloat16

_cache: dict = {}


def _build(mode: str):
    """mode: 'causal' (diag-window masks, upper blocks skipped),
    'none' (no masking, all blocks), 'general' (per-block masks from DRAM)."""
    import concourse.bass as bass
    import concourse.mybir as mybir
    from concourse import bacc
    from concourse.tile import TileContext

    fp32 = mybir.dt.float32
    bf = mybir.dt.bfloat16
    AF = mybir.ActivationFunctionType

    nc = bacc.Bacc("TRN2", target_bir_lowering=False, debug=False,
                   num_devices=N_CORES)

    # host-prepacked inputs (see kernel() below)
    xq = nc.dram_tensor("xq", [NKC, 128, S], bf, kind="ExternalInput")
    xk = nc.dram_tensor("xk", [NKC, 128, S], bf, kind="ExternalInput")
    xv = nc.dram_tensor("xv", [NKC, 128, S], bf, kind="ExternalInput")
    wq = nc.dram_tensor("wq", [128, NKC * GD], bf, kind="ExternalInput")
    wk = nc.dram_tensor("wk", [128, NKC * GD], bf, kind="ExternalInput")
    wv = nc.dram_tensor("wv", [128, NKC * W65], bf, kind="ExternalInput")
    vb = nc.dram_tensor("vb", [128, W65], bf, kind="ExternalInput")
    bqk = nc.dram_tensor("bqk", [128, 4], fp32, kind="ExternalInput")
    if mode == "causal":
        cmw = nc.dram_tensor("cmw", [128, 128], bf, kind="ExternalInput")
    elif mode == "general":
        amaskT = nc.dram_tensor("amaskT", [S, S], bf, kind="ExternalInput")
    out = nc.dram_tensor("out", [HPC, 65, S], fp32, kind="ExternalOutput")

    HS = S // 2  # DMA half

    with TileContext(nc) as tc:
        with (
            tc.tile_pool(name="res", bufs=1) as res,
            tc.tile_pool(name="mload", bufs=4) as mload,
            tc.tile_pool(name="eload", bufs=6) as eload,
            tc.tile_pool(name="oout", bufs=6) as oout,
            tc.tile_pool(name="pproj", bufs=2, space="PSUM") as pproj,
            tc.tile_pool(name="pscore", bufs=2, space="PSUM") as pscore,
            tc.tile_pool(name="ppv", bufs=2, space="PSUM") as ppv,
        ):
            # ---- resident loads: K-path first so PE unblocks fastest ----
            xk_s = res.tile([128, NKC * S], bf, tag="xk")
            xq_s = res.tile([128, NKC * S], bf, tag="xq")
            xv_s = res.tile([128, NKC * S], bf, tag="xv")
            wq_s = res.tile([128, NKC * GD], bf, tag="wq")
            wk_s = res.tile([128, NKC * GD], bf, tag="wk")
            wv_s = res.tile([128, NKC * W65], bf, tag="wv")
            vb_s = res.tile([128, W65], bf, tag="vb")
            bqk_s = res.tile([128, 4], fp32, tag="bqk")

            def xpart(dst, src, kc, lo, hi):
                nc.sync.dma_start(dst[:, kc * S + lo: kc * S + hi],
                                  src[kc, :, lo:hi])

            # first K-proj group needs only wk m=0 (cols 0..1024 in m-major
            # packing) and the first q-chunk of every xk k-chunk
            nc.sync.dma_start(wk_s[:, :NKC * 128], wk[:, :NKC * 128])
            nc.sync.dma_start(bqk_s[:], bqk[:, :])
            for kc in range(NKC):
                xpart(xk_s, xk, kc, 0, QC)
            nc.sync.dma_start(wk_s[:, NKC * 128:], wk[:, NKC * 128:])
            for kc in range(NKC):
                xpart(xk_s, xk, kc, QC, 2 * QC)
            nc.sync.dma_start(wq_s[:], wq[:, :])
            for kc in range(NKC):
                xpart(xq_s, xq, kc, 0, 2 * QC)
            nc.sync.dma_start(wv_s[:], wv[:, :])
            nc.gpsimd.dma_start(vb_s[:], vb[:, :])
            for kc in range(NKC):
                xpart(xv_s, xv, kc, 0, 2 * QC)
            for kc in range(NKC):
                xpart(xk_s, xk, kc, HS, S)
            for kc in range(NKC):
                xpart(xq_s, xq, kc, HS, S)
            for kc in range(NKC):
                xpart(xv_s, xv, kc, HS, S)
            if mode == "causal":
                cmw_s = res.tile([128, 128], bf, tag="cmw")
                nc.gpsimd.dma_start(cmw_s[:], cmw[:, :])

            # resident projected activations
            qt_s = [res.tile([128, S], bf, tag=f"qt{m}", name=f"qt{m}")
                    for m in range(2)]
            kt_s = [res.tile([128, S], bf, tag=f"kt{m}", name=f"kt{m}")
                    for m in range(2)]
            vh_s = res.tile([128, N_KB * W65], bf, tag="vh")

            def attention(sc, filler=None, fill_rate=1):
                cs = slice(sc * QC, (sc + 1) * QC)
                n_kb = 4 * sc + 4 if mode == "causal" else N_KB

                def fill():
                    if filler is None:
                        return
                    for _ in range(fill_rate):
                        try:
                            next(filler)
                        except StopIteration:
                            return

                def score_exp(p, kb):
                    j = kb - 4 * sc if mode == "causal" else -1
                    t = 128 * j if j > 0 else 0  # trimmed leading cols
                    st = pscore.tile([128, 2 * QC], fp32, tag="s")
                    for b2 in range(2):
                        nc.tensor.matmul(
                            st[:, b2 * QC + t:(b2 + 1) * QC],
                            kt_s[p][b2 * 64:(b2 + 1) * 64,
                                    kb * 128:(kb + 1) * 128],
                            qt_s[p][b2 * 64:(b2 + 1) * 64,
                                    sc * QC + t:(sc + 1) * QC],
                            start=True, stop=True)
                    et = eload.tile([128, 2 * QC], bf, tag="e")
                    if j >= 0:
                        for b2 in range(2):
                            nc.vector.tensor_add(
                                st[:, b2 * QC + t: b2 * QC + t + 128],
                                st[:, b2 * QC + t: b2 * QC + t + 128],
                                cmw_s[:])
                    if t > 0:
                        st3 = st[:].rearrange("p (h n) -> p h n", h=2)
                        et3 = et[:].rearrange("p (h n) -> p h n", h=2)
                        nc.scalar.activation(et3[:, :, t:], st3[:, :, t:],
                                             AF.Exp)
                    else:
                        if mode == "general":
                            mt = mload.tile([128, QC], bf, tag="mt")
                            nc.sync.dma_start(
                                mt[:], amaskT[kb * 128:(kb + 1) * 128, cs])
                            for b2 in range(2):
                                nc.vector.tensor_add(
                                    st[:, b2 * QC:(b2 + 1) * QC],
                                    st[:, b2 * QC:(b2 + 1) * QC], mt[:])
                        nc.scalar.activation(et[:], st[:], AF.Exp)
                    return et, t

                def pv_step(p, pv, kb, et, t):
                    for b2 in range(2):
                        h = 2 * p + b2
                        nc.tensor.matmul(
                            pv[b2][:, t:],
                            vh_s[:, kb * W65 + h * 65: kb * W65 + h * 65 + 65],
                            et[:, b2 * QC + t:(b2 + 1) * QC],
                            start=(kb == 0), stop=(kb == n_kb - 1))

                for p in range(2):
                    pv = [ppv.tile([65, QC], fp32, tag="pv", name=f"pv{b2}")
                          for b2 in range(2)]
                    # kb-pair software pipeline: scores for both kbs
                    # back-to-back (hides LDWEIGHTS), then both PV steps
                    for kb2 in range(n_kb // 2):
                        e0, t0 = score_exp(p, 2 * kb2)
                        e1, t1 = score_exp(p, 2 * kb2 + 1)
                        pv_step(p, pv, 2 * kb2, e0, t0)
                        fill()
                        pv_step(p, pv, 2 * kb2 + 1, e1, t1)
                        fill()
                    for b2 in range(2):
                        h = 2 * p + b2
                        ot = oout.tile([65, QC], fp32, tag="o")
                        nc.vector.tensor_copy(ot[:], pv[b2][:])
                        nc.gpsimd.dma_start(out[h, :, cs], ot[:])

            def proj_kq(scp):
                """Generator: K/Q projections for chunks 2scp, 2scp+1 (two
                seq-chunks per weight load), yielding after every matmul."""
                sc0, sc1 = 2 * scp, 2 * scp + 1
                for w_s, x_s, dst, bcol in ((wk_s, xk_s, kt_s, 2),
                                            (wq_s, xq_s, qt_s, 0)):
                    for m in range(2):
                        psA = pproj.tile([128, QC], fp32, tag="proj",
                                         name="psA")
                        psB = pproj.tile([128, QC], fp32, tag="proj",
                                         name="psB")
                        for kc in range(NKC):
                            wsl = w_s[:, m * NKC * 128 + kc * 128:
                                      m * NKC * 128 + (kc + 1) * 128]
                            nc.tensor.matmul(
                                psA[:], wsl,
                                x_s[:, kc * S + sc0 * QC:
                                       kc * S + (sc0 + 1) * QC],
                                start=(kc == 0), stop=(kc == NKC - 1))
                            yield
                            nc.tensor.matmul(
                                psB[:], wsl,
                                x_s[:, kc * S + sc1 * QC:
                                       kc * S + (sc1 + 1) * QC],
                                start=(kc == 0), stop=(kc == NKC - 1))
                            yield
                        bias = bqk_s[:, bcol + m:bcol + m + 1]
                        nc.vector.tensor_scalar_add(
                            dst[m][:, sc0 * QC:(sc0 + 1) * QC], psA[:], bias)
                        nc.vector.tensor_scalar_add(
                            dst[m][:, sc1 * QC:(sc1 + 1) * QC], psB[:], bias)
                        yield

            def proj_v(sc):
                """Generator: V projection for chunk sc, yield per matmul."""
                for sb in range(sc * 4, (sc + 1) * 4):
                    so = sb * 128
                    ps = pproj.tile([128, W65], fp32, tag="proj")
                    for kc in range(NKC):
                        nc.tensor.matmul(
                            ps[:],
                            xv_s[:, kc * S + so: kc * S + so + 128],
                            wv_s[:, kc * W65:(kc + 1) * W65],
                            start=(kc == 0), stop=(kc == NKC - 1))
                        yield
                    nc.vector.tensor_add(vh_s[:, sb * W65:(sb + 1) * W65],
                                         ps[:], vb_s[:])
                    yield

            def chain(*gens):
                for g in gens:
                    yield from g

            def drain(g):
                for _ in g:
                    pass

            # chunks 0/1 projected eagerly; K/Q of chunks 2/3 and V of chunk 2
            # trickle into the attention of chunks 0/1 (one matmul per PV
            # step) to fill PE stalls caused by exp latency; V of chunk 3
            # trickles into attention 2 (its VH only needed by attention 3).
            drain(chain(proj_kq(0), proj_v(0), proj_v(1)))
            g1 = chain(proj_kq(1), proj_v(2))
            attention(0, filler=g1)
            attention(1, filler=g1)
            drain(g1)
            g2 = proj_v(3)
            attention(2, filler=g2)
            drain(g2)
            attention(3)

    nc.compile()
    return nc


def _get_nc(mode: str):
    if mode not in _cache:
        _cache[mode] = _build(mode)
    return _cache[mode]


def kernel(q, k, v, mask, Wq, bq, Wk, bk, Wv, bv):
    q = np.asarray(q, np.float32)
    k = np.asarray(k, np.float32)
    v = np.asarray(v, np.float32)
    Wq = np.asarray(Wq, np.float32)
    Wk = np.asarray(Wk, np.float32)
    Wv = np.asarray(Wv, np.float32)
    bq = np.asarray(bq, np.float32)
    bk = np.asarray(bk, np.float32)
    bv = np.asarray(bv, np.float32)
    m2 = np.asarray(mask)[0, 0]

    causal = bool(np.array_equal(m2 != 0, np.tril(np.ones((S, S), bool))))
    if causal:
        mode = "causal"
    elif np.all(m2 != 0):
        mode = "none"
    else:
        mode = "general"

    from concourse.bass_utils import run_bass_kernel_spmd

    nc = _get_nc(mode)

    in_maps = []
    for c in range(N_CORES):
        b, g = divmod(c, HPC)
        gsl = slice(g * GD, (g + 1) * GD)
        # V weights: per head 65 cols (64 data + zero col for the ones dim);
        # the ones + bias come from the broadcast add tile vb.
        wv65 = np.zeros((D, W65), np.float32)
        vbrow = np.zeros((1, W65), np.float32)
        for h in range(HPC):
            wv65[:, h * 65:h * 65 + 64] = Wv[g * GD + h * DK:
                                             g * GD + h * DK + DK, :].T
            vbrow[0, h * 65:h * 65 + 64] = bv[g * GD + h * DK:
                                              g * GD + h * DK + DK]
            vbrow[0, h * 65 + 64] = 1.0

        def packw(wt):
            n = wt.shape[1]
            return np.ascontiguousarray(
                wt.reshape(NKC, 128, n).transpose(1, 0, 2).reshape(128, NKC * n)
            ).astype(bf16)

        def packw_mmajor(wt):
            # [D, 256] -> [128, (m, kc, 128)] so each m-half is contiguous
            return np.ascontiguousarray(
                wt.reshape(NKC, 128, 2, 128).transpose(1, 2, 0, 3)
                  .reshape(128, NKC * GD)).astype(bf16)

        im = {
            "xq": np.ascontiguousarray(q[b].T.reshape(NKC, 128, S)).astype(bf16),
            "xk": np.ascontiguousarray(k[b].T.reshape(NKC, 128, S)).astype(bf16),
            "xv": np.ascontiguousarray(v[b].T.reshape(NKC, 128, S)).astype(bf16),
            "wq": packw_mmajor(Wq[gsl, :].T / 8.0),
            "wk": packw_mmajor(Wk[gsl, :].T),
            "wv": packw(wv65),
            "vb": np.broadcast_to(vbrow, (128, W65)).astype(bf16).copy(),
            "bqk": np.stack([bq[gsl][:128] / 8.0, bq[gsl][128:] / 8.0,
                             bk[gsl][:128], bk[gsl][128:]], 1)
                     .astype(np.float32).copy(),
        }
        if mode == "causal":
            r = np.arange(128)[:, None]
            cc = np.arange(128)[None, :]
            im["cmw"] = np.where(r <= cc, 0.0, -1e9).astype(bf16)
        elif mode == "general":
            add = np.where(m2 == 0, -1e9, 0.0).astype(np.float32)
            im["amaskT"] = add.T.astype(bf16).copy()
        in_maps.append(im)

    global _last_in_maps
    _last_in_maps = in_maps
    res = run_bass_kernel_spmd(nc, in_maps, core_ids=list(range(N_CORES)))

    outf = np.empty((B, S, D), np.float32)
    for c in range(N_CORES):
        b, g = divmod(c, HPC)
        o = res.results[c]["out"]  # [HPC, 65, S]
        num = o[:, :64, :]         # [HPC, 64, S]
        den = o[:, 64:65, :]       # [HPC, 1, S]
        oh = num / den             # [HPC, 64, S]
        outf[b, :, g * GD:(g + 1) * GD] = (
            oh.transpose(2, 0, 1).reshape(S, GD))
    return outf
